# revision 28
# baseline (speedup 1.0000x reference)
"""Trainium2 Bass kernel for MineralDepositGCN (3x GCNConv+BN + MLP head).

Strategy (8 NeuronCores, SPMD single program), per the sharding hint:
  - Nodes sharded by range: core c owns nodes [c*12500, (c+1)*12500),
    padded to NPAD=12800. Edges owned by their DST core, so aggregation is
    device-local; the halo exchange is an AllGather of projected features.
  - Per layer: project own shard h@W into a compact node-major f32 table
    [NPAD, 64] (256B rows), AllGather -> [8*NPAD, 64] in DRAM.
  - Messages fetched with dma_gather (256B rows, all-useful f32). int16
    gather indices reach 32767 rows only, so edges are bucketed by
    src-core-pair "window" (4 windows x 25600 rows), gathered from a
    sliced table view.
  - Scatter-add via dma_scatter_add (SDMA CCE f32 accumulate) into a
    DRAM aggregate [NPAD, 64]. CCE loses concurrent duplicate updates
    within one call, so edges are split into conflict-free runs: run
    (w, k) holds each dst's k-th edge from window w; runs execute as
    separate (serialized) scatter calls. Gathers batch several runs per
    call; edge-weight scaling is one in-place broadcast multiply per
    segment. This keeps the whole edge stage at ~260 instructions/layer
    (the backend executes ~12k instructions/s, so instruction count
    dominates the runtime).
  - Aggregate drain: f32 node-major -> bf16 [6400, 128] (two nodes per
    row) -> hardware xbar dma transpose -> parity-interleaved
    feature-major h [128=(feat,parity), 6400]. Projection/BN/MLP all
    operate on this layout at unchanged instruction counts.
  - BN stats via 2 accum passes + parity fold + tiny AllReduce with
    closed-form pad-row correction.
"""
import os
import numpy as np
import ml_dtypes

from concourse import bass, bacc, tile, mybir
from concourse import bass_utils
from concourse.bass_interp import get_hw_module

BF16 = mybir.dt.bfloat16
F32 = mybir.dt.float32
I16 = mybir.dt.int16
ALU = mybir.AluOpType
ACTF = mybir.ActivationFunctionType

NCORES = 8
EPS = 1e-5
SEGMAX = 13056          # max gather-segment slots (128-mult)
SCATMAX = 8064          # max slots per dma_scatter_add call (2 desc/idx, <16384)


def _cfg(n_nodes, in_c, hid, ncls):
    shard = n_nodes // NCORES
    npad = ((shard + 511) // 512) * 512
    return dict(
        N=n_nodes, IN_C=in_c, HID=hid, NCLS=ncls,
        SHARD=shard, NPAD=npad,
        NTILES=npad // 128,
        NTOT=npad * NCORES,
        WINR=2 * npad,
        NWIN=NCORES // 2,
    )


def _plan(edge_index, cfg):
    """Conflict-free run structure shared by all cores.

    Edges keyed by (window w, rank k) where k = occurrence index of the
    edge's dst within window w on its owner core. Run (w, k) has
    RL[w][k] = 128*ceil(max_core count/128) slots. Runs are packed in
    (w, k) order into gather segments of <= SEGMAX slots.
    """
    SHARD, NWIN, NPAD = cfg["SHARD"], cfg["NWIN"], cfg["NPAD"]
    src = edge_index[0].astype(np.int64)
    dst = edge_index[1].astype(np.int64)
    d_owner = np.minimum(dst // SHARD, NCORES - 1)
    d_local = dst - d_owner * SHARD
    s_owner = np.minimum(src // SHARD, NCORES - 1)
    w = s_owner // 2

    # rank of each edge within its (owner, w, dst) group
    key = (d_owner * NWIN + w) * SHARD + d_local
    order = np.argsort(key, kind="stable")
    sk = key[order]
    new = np.ones(len(sk), dtype=bool)
    new[1:] = sk[1:] != sk[:-1]
    idxs = np.arange(len(sk))
    starts = idxs[new]
    grp_start = starts[np.cumsum(new) - 1]
    rank_sorted = idxs - grp_start
    rank = np.empty(len(sk), dtype=np.int64)
    rank[order] = rank_sorted

    KMAX = int(rank.max()) + 1
    cnt = np.zeros((NCORES, NWIN, KMAX), dtype=np.int64)
    np.add.at(cnt.reshape(-1), (d_owner * NWIN + w) * KMAX + rank, 1)
    cmax = cnt.max(axis=0)                       # [NWIN, KMAX]
    RL = ((cmax + 127) // 128) * 128
    run_off = np.zeros((NWIN, KMAX), dtype=np.int64)
    segments = []                                # per window
    off = 0
    for wi in range(NWIN):
        segs = []
        cur_off, cur_n, cur_runs = off, 0, []
        for k in range(KMAX):
            rl = int(RL[wi, k])
            if rl == 0:
                continue
            if cur_n + rl > SEGMAX and cur_n > 0:
                segs.append((cur_off, cur_n, cur_runs))
                cur_off, cur_n, cur_runs = off, 0, []
            run_off[wi, k] = off
            cur_runs.append(k)
            cur_n += rl
            off += rl
        if cur_n > 0:
            segs.append((cur_off, cur_n, cur_runs))
        segments.append(segs)
    NSLOT = int(off)
    return dict(rank=rank, w=w, d_owner=d_owner, d_local=d_local,
                RL=RL, run_off=run_off, segments=segments,
                NSLOT=NSLOT, KMAX=KMAX)


def _preprocess(x, edge_index, edge_attr, cfg, plan):
    SHARD, NPAD, NWIN = cfg["SHARD"], cfg["NPAD"], cfg["NWIN"]
    WINR = cfg["WINR"]
    NSLOT = plan["NSLOT"]
    src = edge_index[0].astype(np.int64)
    ew = np.asarray(edge_attr, dtype=np.float32)
    s_owner = np.minimum(src // SHARD, NCORES - 1)
    s_local = src - s_owner * SHARD
    gidx_all = (s_owner * NPAD + s_local) - plan["w"] * WINR
    AGGR = NPAD + 128
    JUNK = NPAD                                  # scatter junk row

    bf = ml_dtypes.bfloat16
    per_core = []
    for c in range(NCORES):
        m = plan["d_owner"] == c
        wi = plan["w"][m]
        k = plan["rank"][m]
        gi = gidx_all[m]
        dl = plan["d_local"][m]
        we = ew[m]
        # position within run: order by (w, k, dst)
        order = np.lexsort((dl, k, wi))
        wi, k, gi, dl, we = (wi[order], k[order], gi[order], dl[order],
                             we[order])
        runkey = wi * plan["KMAX"] + k
        new = np.ones(len(runkey), dtype=bool)
        new[1:] = runkey[1:] != runkey[:-1]
        idxs = np.arange(len(runkey))
        starts = idxs[new]
        pos = idxs - starts[np.cumsum(new) - 1]
        slot = plan["run_off"][wi, k] + pos

        gidx16 = np.zeros(NSLOT, dtype=np.int16)
        sidx16 = np.full(NSLOT, JUNK, dtype=np.int16)
        ewb = np.zeros(NSLOT, dtype=np.float32)
        gidx16[slot] = gi.astype(np.int16)
        sidx16[slot] = (dl + (k % 2) * AGGR).astype(np.int16)
        ewb[slot] = we

        # combined per-segment blocks: [gather idx | scatter idx]
        cidx = np.zeros(2 * NSLOT, dtype=np.int16)
        for segs in plan["segments"]:
            for (soff, nsl, _runs) in segs:
                cidx[2 * soff:2 * soff + nsl] = gidx16[soff:soff + nsl]
                cidx[2 * soff + nsl:2 * (soff + nsl)] = \
                    sidx16[soff:soff + nsl]

        per_core.append(dict(
            c_idx=cidx.reshape(2 * NSLOT // 16, 16).T.copy(),
            ew_s=ewb.reshape(NSLOT // 128, 128).T.astype(bf),
        ))

    for c in range(NCORES):
        xs = np.zeros((cfg["IN_C"], NPAD), dtype=np.float32)
        xs[:, :SHARD] = np.asarray(x[c * SHARD:(c + 1) * SHARD]).T
        per_core[c]["x_t"] = xs.astype(bf)
    return per_core


def _build(cfg, plan):
    IN_C, HID, NCLS = cfg["IN_C"], cfg["HID"], cfg["NCLS"]
    NPAD, NTILES = cfg["NPAD"], cfg["NTILES"]
    NTOT, WINR, NWIN = cfg["NTOT"], cfg["WINR"], cfg["NWIN"]
    NSLOT = plan["NSLOT"]
    RL, run_off, segments = plan["RL"], plan["run_off"], plan["segments"]
    NPAIR = NPAD // 2
    AGGR = NPAD + 128
    NPADDING = float(NCORES * NPAD - cfg["N"])
    INVN = 1.0 / cfg["N"]
    SEGC = SEGMAX // 128
    LVL = int(os.environ.get("KLEVEL", "0"))
    NCH = NSLOT // 128

    nc = bacc.Bacc("TRN2", target_bir_lowering=False, debug=False,
                   num_devices=NCORES)

    def din(name, shape, dt):
        return nc.dram_tensor(name, shape, dt, kind="ExternalInput").ap()

    x_t_d = din("x_t", [IN_C, NPAD], BF16)
    cidx_d = din("c_idx", [16, 2 * NSLOT // 16], I16)
    ew_d = din("ew_s", [128, NCH], BF16)
    # packed weights: wb = bf16 [128, 64+64+64+128+64+5=389]
    # (cw0 | cw1dup | cw2dup | mw1dup | mw2 | mw3)
    # wf = f32 [128, 16]: cols 0-5 pad_corr(x3 layers)+junk rows, 6-8 cbd,
    # 9-11 bng, 12-14 bnb (HID rows), 15: mb1(128) ; mb2/mb3 packed in
    # rows of col 15? -> keep separate small cols; see host packing.
    wb_d = din("w_bf", [128, 389], BF16)
    wf_d = din("w_f32", [128, 20], F32)
    out_d = nc.dram_tensor("out5", [NCLS, NPAD], F32,
                           kind="ExternalOutput").ap()
    dbg = bool(os.environ.get("KDBG"))
    if dbg:
        dbg_tab = nc.dram_tensor("dbg_tab", [NTOT, 64], F32,
                                 kind="ExternalOutput").ap()
        dbg_agg = nc.dram_tensor("dbg_agg", [NPAD, 64], F32,
                                 kind="ExternalOutput").ap()
        dbg_h = nc.dram_tensor("dbg_h", [128, NPAIR], F32,
                               kind="ExternalOutput").ap()

    rg = [list(range(NCORES))]

    with tile.TileContext(nc) as tc:
        with tc.tile_pool(name="sb", bufs=1) as sb, \
             tc.tile_pool(name="sb2", bufs=2) as sb2, \
             tc.tile_pool(name="hxp", bufs=2) as hxp, \
             tc.tile_pool(name="drp", bufs=1) as drp, \
             tc.tile_pool(name="mp", bufs=2) as mp, \
             tc.tile_pool(name="idxp", bufs=2) as idxp, \
             tc.tile_pool(name="psP", bufs=2, space="PSUM") as psP, \
             tc.tile_pool(name="psM", bufs=1, space="PSUM") as psM, \
             tc.tile_pool(name="dram", bufs=1, space="DRAM") as dram, \
             tc.tile_pool(name="dram2", bufs=2, space="DRAM") as dram2:

            # ---- persistent loads ----
            ew_f = sb.tile([128, NCH], F32, tag="ewf")
            EWCH = 1664
            for e0 in range(0, NCH, EWCH):
                e1 = min(e0 + EWCH, NCH)
                ewt = sb2.tile([128, EWCH], BF16, tag="ewtmp")
                nc.sync.dma_start(out=ewt[:, 0:e1 - e0], in_=ew_d[:, e0:e1])
                nc.vector.tensor_copy(out=ew_f[:, e0:e1],
                                      in_=ewt[:, 0:e1 - e0])
            wb = sb.tile([128, 389], BF16, tag="wb")
            nc.sync.dma_start(out=wb[:], in_=wb_d[:])
            wf = sb.tile([128, 20], F32, tag="wf")
            nc.sync.dma_start(out=wf[:], in_=wf_d[:])
            cw_t = [wb[:, 0:64], wb[:, 64:128], wb[:, 128:192]]
            mw1_t = wb[:, 192:320]
            mw2_t = wb[0:2 * HID, 320:384]
            mw3_t = wb[0:HID, 384:389]
            pcorr_t = wf[0:HID, 0:6]
            cb2_t = [wf[:, 6 + l:7 + l] for l in range(3)]
            bng_t = [wf[0:HID, 9 + l:10 + l] for l in range(3)]
            bnb_t = [wf[0:HID, 12 + l:13 + l] for l in range(3)]
            mb1_t = wf[:, 15:16]
            mb2_t = wf[0:HID, 16:17]
            mb3_t = wf[0:NCLS, 17:18]
            eps_t = wf[0:HID, 18:19]
            zt = sb.tile([128, 1664], F32, tag="zero")
            nc.vector.memset(zt[:], 0.0)

            x_t = sb.tile([IN_C, NPAD], BF16, tag="hx0")
            nc.sync.dma_start(out=x_t[:], in_=x_t_d[:])

            # replicate wrapped indices [16, X] -> [128, X] in DRAM
            cidx_rep = dram.tile([128, 2 * NSLOT // 16], I16, tag="cidxrep")
            if LVL < 3:
                for r in range(8):
                    nc.sync.dma_start(
                        out=cidx_rep[16 * r:16 * (r + 1), :], in_=cidx_d[:])
            # pre-zeroed DRAM block to reset the aggregate each layer
            zdram = dram.tile([2 * AGGR, 64], F32, tag="zdram")
            zv = zdram[:].rearrange("(k p) f -> p k f", p=128)
            NZCH = 2 * AGGR // 128
            ZC = 1664 // 64
            for z0 in range(0, NZCH, ZC):
                z1 = min(z0 + ZC, NZCH)
                nc.sync.dma_start(
                    out=zv[:, z0:z1, :],
                    in_=zt[:, 0:(z1 - z0) * 64].rearrange(
                        "p (k f) -> p k f", f=64))

            h_cur = x_t              # layer0: plain feature-major
            for l in range(3):
                cdim = IN_C if l == 0 else HID
                # ---- projection -> compact node-major f32 table ----
                tab_in = dram2.tile([NPAD, 64], F32, tag="tabin")
                PB = 16
                if l == 0:
                    srcs = [(h_cur, 0, NTILES, tab_in[:])]
                else:
                    evens = tab_in[:].rearrange("(q i) f -> q i f", i=2)
                    srcs = [(h_cur, 0, NTILES // 2, evens[:, 0, :]),
                            (h_cur, HID, NTILES // 2, evens[:, 1, :])]
                for hsrc, prow, ntile, tview in srcs:
                    for g in range(0, ntile, PB):
                        gn = min(PB, ntile - g)
                        pp = psP.tile([128, PB * HID], F32, tag="proj",
                                      space="PSUM")
                        for j in range(gn):
                            kk = g + j
                            nc.tensor.matmul(
                                out=pp[:, j * HID:(j + 1) * HID],
                                lhsT=hsrc[prow:prow + cdim,
                                          kk * 128:(kk + 1) * 128],
                                rhs=cw_t[l][prow:prow + cdim, :],
                                start=True, stop=True)
                        stg = sb2.tile([128, PB, HID], F32, tag="stage")
                        nc.any.tensor_copy(
                            out=stg[:, 0:gn, :],
                            in_=pp[:, 0:gn * HID].rearrange(
                                "p (j f) -> p j f", j=gn))
                        tv = tview.rearrange("(g p) f -> g p f", p=128)
                        nc.sync.dma_start(
                            out=tv[g:g + gn].rearrange("g p f -> p g f"),
                            in_=stg[:, 0:gn, :])

                # ---- AllGather the projected table ----
                tab_full = dram2.tile([NTOT, 64], F32, tag="tabfull")
                if LVL < 4:
                    nc.gpsimd.collective_compute(
                        "AllGather", ALU.bypass, replica_groups=rg,
                        ins=[tab_in[:]], outs=[tab_full[:]])

                # ---- zero aggregate, gather + scatter-add ----
                agg = dram2.tile([2 * AGGR, 64], F32, tag="agg")
                nc.sync.dma_start(out=agg[:], in_=zdram[:])
                for wi in range(NWIN):
                    for (soff, nsl, runs) in segments[wi]:
                        segc = nsl // 128
                        if LVL < 3:
                            ci_t = idxp.tile([128, 2 * SEGC * 8], I16,
                                             tag="ci")
                            nc.sync.dma_start(
                                out=ci_t[:, 0:2 * nsl // 16],
                                in_=cidx_rep[:, 2 * soff // 16:
                                             2 * (soff + nsl) // 16])
                            gi_t = ci_t[:, 0:nsl // 16]
                            si_t = ci_t[:, nsl // 16:2 * nsl // 16]
                            m_t = mp.tile([128, SEGC, 64], F32, tag="mt")
                            nc.gpsimd.dma_gather(
                                out_ap=m_t[:, 0:segc, :],
                                in_ap=tab_full[wi * WINR:(wi + 1) * WINR, :],
                                idxs_ap=gi_t,
                                num_idxs=nsl, num_idxs_reg=nsl,
                                elem_size=64, single_packet=False)
                        if LVL < 2:
                            a0, a1 = bass.broadcast_tensor_aps(
                                m_t[:, 0:segc, :],
                                ew_f[:, soff // 128:soff // 128 + segc]
                                .rearrange("p (k a) -> p k a", a=1))
                            nc.vector.tensor_tensor(
                                out=m_t[:, 0:segc, :], in0=a0, in1=a1,
                                op=ALU.mult)
                        if LVL < 1:
                            # pair-merged conflict-free spans (k//2 groups)
                            spans = []
                            for k in runs:
                                ro, rl = int(run_off[wi][k]), int(RL[wi][k])
                                if spans and spans[-1][2] == k // 2 and \
                                        spans[-1][1] == ro:
                                    spans[-1] = (spans[-1][0], ro + rl,
                                                 k // 2)
                                else:
                                    spans.append((ro, ro + rl, k // 2))
                            for (a, b, _pid) in spans:
                                for p0 in range(a, b, SCATMAX):
                                    pn = min(SCATMAX, b - p0)
                                    lo = p0 - soff
                                    nc.gpsimd.dma_scatter_add(
                                        out_ap=agg[:],
                                        in_ap=m_t[:, lo // 128:
                                                  (lo + pn) // 128, :],
                                        idxs_ap=si_t[:, lo // 16:
                                                     (lo + pn) // 16],
                                        num_idxs=pn, num_idxs_reg=pn,
                                        elem_size=64, single_packet=False)

                # ---- drain: f32 node-major -> bf16 pair-rows -> xbar T ----
                if dbg and l == 0:
                    nc.sync.dma_start(out=dbg_tab[:], in_=tab_full[:])
                    nc.sync.dma_start(out=dbg_agg[:], in_=agg[0:NPAD, :])  # even half only
                af = drp.tile([128, NTILES, 64], F32, tag="af")
                av0 = agg[0:NPAD, :].rearrange("(k p) f -> p k f", p=128)
                av1 = agg[AGGR:AGGR + NPAD, :].rearrange(
                    "(k p) f -> p k f", p=128)
                nc.sync.dma_start(out=af[:], in_=av0)
                HT = NTILES // 2
                for hh in range(2):
                    nc.gpsimd.dma_start(
                        out=af[:, hh * HT:(hh + 1) * HT, :],
                        in_=av1[:, hh * HT:(hh + 1) * HT, :],
                        accum_op=ALU.add)
                abf = drp.tile([128, NTILES, 64], BF16, tag="abf")
                nc.vector.tensor_copy(out=abf[:], in_=af[:])
                tmp = dram2.tile([NPAIR, 128], BF16, tag="ptmp")
                nc.sync.dma_start(
                    out=tmp[:].rearrange("(k q) (i f) -> (q i) k f",
                                         q=64, i=2),
                    in_=abf[:])
                h_new = hxp.tile([128, NPAIR], BF16, tag="hx")
                nc.sync.dma_start_transpose(out=h_new[:], in_=tmp[:])
                # bias + relu (bias duplicated across parity halves)
                nc.vector.tensor_scalar(
                    out=h_new[:], in0=h_new[:], scalar1=cb2_t[l],
                    scalar2=0.0, op0=ALU.add, op1=ALU.max)

                # ---- BN stats (global, pad-row corrected) ----
                scr = drp.tile([128, NPAIR], BF16, tag="scr")
                st2 = sb.tile([128, 2], F32, tag="st2")
                nc.scalar.activation(out=scr[:], in_=h_new[:],
                                     func=ACTF.Copy, accum_out=st2[:, 0:1])
                nc.scalar.activation(out=scr[:], in_=h_new[:],
                                     func=ACTF.Square, accum_out=st2[:, 1:2])
                fold = sb.tile([HID, 2], F32, tag="fold")
                nc.sync.dma_start(out=fold[:], in_=st2[HID:128, :])
                sc = sb.tile([HID, 2], F32, tag="statsc")
                nc.vector.tensor_tensor(out=sc[:], in0=pcorr_t[:, 2 * l:
                                                              2 * l + 2],
                                        in1=st2[0:HID, :], op=ALU.add)
                nc.vector.tensor_tensor(out=sc[:], in0=sc[:],
                                        in1=fold[:], op=ALU.add)
                stats_in = dram.tile([HID, 2], F32, tag=f"stin{l}")
                stats_out = dram.tile([HID, 2], F32, tag=f"stout{l}")
                st_sb = sb.tile([HID, 2], F32, tag="stsb")
                if LVL < 4:
                    nc.sync.dma_start(out=stats_in[:], in_=sc[:])
                    nc.gpsimd.collective_compute(
                        "AllReduce", ALU.add, replica_groups=rg,
                        ins=[stats_in[:]], outs=[stats_out[:]])
                    nc.sync.dma_start(out=st_sb[:], in_=stats_out[:])
                else:
                    nc.vector.tensor_copy(out=st_sb[:], in_=sc[:])
                mt = sb.tile([HID, 1], F32, tag="mt1")
                nc.vector.tensor_scalar(out=mt[:], in0=st_sb[:, 0:1],
                                        scalar1=INVN, scalar2=None,
                                        op0=ALU.mult)
                vt = sb.tile([HID, 1], F32, tag="vt")
                nc.vector.tensor_scalar(out=vt[:], in0=st_sb[:, 1:2],
                                        scalar1=INVN, scalar2=None,
                                        op0=ALU.mult)
                msq = sb.tile([HID, 1], F32, tag="msq")
                nc.vector.tensor_tensor(out=msq[:], in0=mt[:], in1=mt[:],
                                        op=ALU.mult)
                nc.vector.tensor_tensor(out=vt[:], in0=vt[:], in1=msq[:],
                                        op=ALU.subtract)
                sqv = sb.tile([HID, 1], F32, tag="sqv")
                nc.scalar.activation(out=sqv[:], in_=vt[:], func=ACTF.Sqrt,
                                     bias=eps_t)
                rstd = sb.tile([HID, 1], F32, tag="rstd")
                nc.vector.reciprocal(out=rstd[:], in_=sqv[:])
                s_t = sb.tile([128, 1], F32, tag="sT")
                nc.vector.tensor_tensor(out=s_t[0:HID, :], in0=bng_t[l],
                                        in1=rstd[:], op=ALU.mult)
                t_t = sb.tile([128, 1], F32, tag="tT")
                nc.vector.tensor_tensor(out=t_t[0:HID, :], in0=mt[:],
                                        in1=s_t[0:HID, :], op=ALU.mult)
                nc.vector.tensor_tensor(out=t_t[0:HID, :], in0=bnb_t[l],
                                        in1=t_t[0:HID, :], op=ALU.subtract)
                nc.sync.dma_start(out=s_t[HID:128, :], in_=s_t[0:HID, :])
                nc.sync.dma_start(out=t_t[HID:128, :], in_=t_t[0:HID, :])
                nc.vector.tensor_scalar(out=h_new[:], in0=h_new[:],
                                        scalar1=s_t[:], scalar2=t_t[:],
                                        op0=ALU.mult, op1=ALU.add)
                if dbg and l == 0:
                    dh1 = drp.tile([128, NPAIR], F32, tag="dh1")
                    nc.vector.tensor_copy(out=dh1[:], in_=h_new[:])
                    nc.sync.dma_start(out=dbg_h[:], in_=dh1[:])
                h_cur = h_new

            # ---- MLP head (parity-interleaved feature-major) ----
            MB = 512
            ov = out_d[:].rearrange("c (q i) -> c q i", i=2)
            for par in range(2):
                hsec = h_cur[par * HID:(par + 1) * HID, :]
                for s0 in range(0, NPAIR, MB):
                    s1 = min(s0 + MB, NPAIR)
                    n = s1 - s0
                    p1 = psM.tile([2 * HID, MB], F32, tag="mp1", space="PSUM")
                    nc.tensor.matmul(out=p1[:, 0:n],
                                     lhsT=mw1_t[par * HID:(par + 1) * HID, :],
                                     rhs=hsec[:, s0:s1],
                                     start=True, stop=True)
                    a1 = sb2.tile([2 * HID, MB], BF16, tag="a1")
                    nc.scalar.activation(out=a1[:, 0:n], in_=p1[:, 0:n],
                                         func=ACTF.Gelu, bias=mb1_t)
                    p2 = psM.tile([HID, MB], F32, tag="mp2", space="PSUM")
                    nc.tensor.matmul(out=p2[:, 0:n], lhsT=mw2_t,
                                     rhs=a1[:, 0:n], start=True, stop=True)
                    a2 = sb2.tile([HID, MB], BF16, tag="a2")
                    nc.scalar.activation(out=a2[:, 0:n], in_=p2[:, 0:n],
                                         func=ACTF.Gelu, bias=mb2_t)
                    p3 = psM.tile([NCLS, MB], F32, tag="mp3", space="PSUM")
                    nc.tensor.matmul(out=p3[:, 0:n], lhsT=mw3_t,
                                     rhs=a2[:, 0:n], start=True, stop=True)
                    ob = sb2.tile([NCLS, MB], F32, tag="ob")
                    nc.vector.tensor_scalar(out=ob[:, 0:n], in0=p3[:, 0:n],
                                            scalar1=mb3_t, scalar2=None,
                                            op0=ALU.add)
                    nc.sync.dma_start(out=ov[:, s0:s1, par],
                                      in_=ob[:, 0:n])
    nc.compile()
    return nc


def kernel(x, edge_index, edge_attr,
           conv_w0, conv_b0, conv_w1, conv_b1, conv_w2, conv_b2,
           bn_g0, bn_be0, bn_g1, bn_be1, bn_g2, bn_be2,
           mlp_w1, mlp_b1, mlp_w2, mlp_b2, mlp_w3, mlp_b3):
    x = np.asarray(x)
    N, in_c = x.shape
    hid = np.asarray(conv_w0).shape[1]
    ncls = np.asarray(mlp_w3).shape[1]
    cfg = _cfg(N, in_c, hid, ncls)

    edge_index = np.asarray(edge_index)
    plan = _plan(edge_index, cfg)
    per_core = _preprocess(x, edge_index, edge_attr, cfg, plan)

    bf = ml_dtypes.bfloat16
    EPS_ = EPS
    wbp = np.zeros((128, 389), np.float32)
    wbp[:, 0:64] = np.asarray(conv_w0)
    wbp[0:hid, 64:128] = np.asarray(conv_w1)
    wbp[hid:128, 64:128] = np.asarray(conv_w1)
    wbp[0:hid, 128:192] = np.asarray(conv_w2)
    wbp[hid:128, 128:192] = np.asarray(conv_w2)
    wbp[0:hid, 192:320] = np.asarray(mlp_w1)
    wbp[hid:128, 192:320] = np.asarray(mlp_w1)
    wbp[0:2 * hid, 320:384] = np.asarray(mlp_w2)
    wbp[0:hid, 384:389] = np.asarray(mlp_w3)
    npadding = float(NCORES * cfg["NPAD"] - N)
    wfp = np.zeros((128, 20), np.float32)
    for l, (cb, bg, be) in enumerate([(conv_b0, bn_g0, bn_be0),
                                      (conv_b1, bn_g1, bn_be1),
                                      (conv_b2, bn_g2, bn_be2)]):
        cb = np.asarray(cb, dtype=np.float32)
        pb = np.maximum(cb, 0.0)
        wfp[0:hid, 2 * l] = -npadding * pb
        wfp[0:hid, 2 * l + 1] = -npadding * pb * pb
        wfp[0:hid, 6 + l] = cb
        wfp[hid:128, 6 + l] = cb
        wfp[0:hid, 9 + l] = np.asarray(bg)
        wfp[0:hid, 12 + l] = np.asarray(be)
    wfp[:, 15] = np.asarray(mlp_b1, dtype=np.float32)
    wfp[0:hid, 16] = np.asarray(mlp_b2, dtype=np.float32)
    wfp[0:ncls, 17] = np.asarray(mlp_b3, dtype=np.float32)
    wfp[0:hid, 18] = EPS_
    common = dict(w_bf=wbp.astype(bf), w_f32=wfp)
    in_maps = []
    for c in range(NCORES):
        m = dict(common)
        m["x_t"] = per_core[c]["x_t"]
        m["c_idx"] = per_core[c]["c_idx"]
        m["ew_s"] = per_core[c]["ew_s"]
        in_maps.append(m)

    nc = _build(cfg, plan)
    nc.m = get_hw_module(nc.m)
    res = bass_utils.run_bass_kernel_spmd(
        nc, in_maps, core_ids=list(range(NCORES)))

    kernel._last_res = res
    kernel._last_nc = nc
    kernel._last_in_maps = in_maps
    out = np.empty((N, cfg["NCLS"]), dtype=np.float32)
    SHARD = cfg["SHARD"]
    for c in range(NCORES):
        out[c * SHARD:(c + 1) * SHARD] = res.results[c]["out5"][:, :SHARD].T
    return out


# revision 29
# speedup vs baseline: 6.6536x; 6.6536x over previous
"""Trainium2 Bass kernel for MineralDepositGCN (3x GCNConv+BN + MLP head).

Strategy (8 NeuronCores, SPMD single program), per the sharding hint:
  - Nodes sharded by range: core c owns nodes [c*12500, (c+1)*12500),
    padded to NPAD=12800. Edges owned by their DST core, so aggregation is
    device-local; the halo exchange is an AllGather of projected features.
  - Per layer: project own shard h@W into a compact node-major f32 table
    [NPAD, 64] (256B rows), AllGather -> [8*NPAD, 64] in DRAM.
  - Messages fetched with dma_gather (256B rows, all-useful f32). int16
    gather indices reach 32767 rows only, so edges are bucketed by
    src-core-pair "window" (4 windows x 25600 rows), gathered from a
    sliced table view.
  - Scatter-add via dma_scatter_add (SDMA CCE f32 accumulate) into a
    DRAM aggregate [NPAD, 64]. CCE loses concurrent duplicate updates
    within one call, so edges are split into conflict-free runs: run
    (w, k) holds each dst's k-th edge from window w; runs execute as
    separate (serialized) scatter calls. Gathers batch several runs per
    call; edge-weight scaling is one in-place broadcast multiply per
    segment. This keeps the whole edge stage at ~260 instructions/layer
    (the backend executes ~12k instructions/s, so instruction count
    dominates the runtime).
  - Aggregate drain: f32 node-major -> bf16 [6400, 128] (two nodes per
    row) -> hardware xbar dma transpose -> parity-interleaved
    feature-major h [128=(feat,parity), 6400]. Projection/BN/MLP all
    operate on this layout at unchanged instruction counts.
  - BN stats via 2 accum passes + parity fold + tiny AllReduce with
    closed-form pad-row correction.
"""
import os
import numpy as np
import ml_dtypes

from concourse import bass, bacc, tile, mybir
from concourse import bass_utils
from concourse.bass_interp import get_hw_module

BF16 = mybir.dt.bfloat16
F32 = mybir.dt.float32
I16 = mybir.dt.int16
ALU = mybir.AluOpType
ACTF = mybir.ActivationFunctionType

NCORES = 8
EPS = 1e-5
SEGMAX = 13056          # max gather-segment slots (128-mult)
SCATMAX = 8064          # max slots per dma_scatter_add call (2 desc/idx, <16384)


def _cfg(n_nodes, in_c, hid, ncls):
    shard = n_nodes // NCORES
    npad = ((shard + 511) // 512) * 512
    return dict(
        N=n_nodes, IN_C=in_c, HID=hid, NCLS=ncls,
        SHARD=shard, NPAD=npad,
        NTILES=npad // 128,
        NTOT=npad * NCORES,
        WINR=2 * npad,
        NWIN=NCORES // 2,
    )


def _plan(edge_index, cfg):
    """Conflict-free run structure shared by all cores.

    Edges keyed by (window w, rank k) where k = occurrence index of the
    edge's dst within window w on its owner core. Run (w, k) has
    RL[w][k] = 128*ceil(max_core count/128) slots. Runs are packed in
    (w, k) order into gather segments of <= SEGMAX slots.
    """
    SHARD, NWIN, NPAD = cfg["SHARD"], cfg["NWIN"], cfg["NPAD"]
    src = edge_index[0].astype(np.int64)
    dst = edge_index[1].astype(np.int64)
    d_owner = np.minimum(dst // SHARD, NCORES - 1)
    d_local = dst - d_owner * SHARD
    s_owner = np.minimum(src // SHARD, NCORES - 1)
    w = s_owner // 2

    # rank of each edge within its (owner, w, dst) group
    key = (d_owner * NWIN + w) * SHARD + d_local
    order = np.argsort(key, kind="stable")
    sk = key[order]
    new = np.ones(len(sk), dtype=bool)
    new[1:] = sk[1:] != sk[:-1]
    idxs = np.arange(len(sk))
    starts = idxs[new]
    grp_start = starts[np.cumsum(new) - 1]
    rank_sorted = idxs - grp_start
    rank = np.empty(len(sk), dtype=np.int64)
    rank[order] = rank_sorted

    KMAX = int(rank.max()) + 1
    cnt = np.zeros((NCORES, NWIN, KMAX), dtype=np.int64)
    np.add.at(cnt.reshape(-1), (d_owner * NWIN + w) * KMAX + rank, 1)
    cmax = cnt.max(axis=0)                       # [NWIN, KMAX]
    RL = ((cmax + 127) // 128) * 128
    run_off = np.zeros((NWIN, KMAX), dtype=np.int64)
    segments = []                                # per window
    off = 0
    for wi in range(NWIN):
        segs = []
        cur_off, cur_n, cur_runs = off, 0, []
        for k in range(KMAX):
            rl = int(RL[wi, k])
            if rl == 0:
                continue
            if cur_n + rl > SEGMAX and cur_n > 0:
                segs.append((cur_off, cur_n, cur_runs))
                cur_off, cur_n, cur_runs = off, 0, []
            run_off[wi, k] = off
            cur_runs.append(k)
            cur_n += rl
            off += rl
        if cur_n > 0:
            segs.append((cur_off, cur_n, cur_runs))
        segments.append(segs)
    NSLOT = int(off)
    return dict(rank=rank, w=w, d_owner=d_owner, d_local=d_local,
                RL=RL, run_off=run_off, segments=segments,
                NSLOT=NSLOT, KMAX=KMAX)


def _preprocess(x, edge_index, edge_attr, cfg, plan):
    SHARD, NPAD, NWIN = cfg["SHARD"], cfg["NPAD"], cfg["NWIN"]
    WINR = cfg["WINR"]
    NSLOT = plan["NSLOT"]
    src = edge_index[0].astype(np.int64)
    ew = np.asarray(edge_attr, dtype=np.float32)
    s_owner = np.minimum(src // SHARD, NCORES - 1)
    s_local = src - s_owner * SHARD
    gidx_all = (s_owner * NPAD + s_local) - plan["w"] * WINR
    AGGR = NPAD + 128
    JUNK = NPAD                                  # scatter junk row

    bf = ml_dtypes.bfloat16
    per_core = []
    for c in range(NCORES):
        m = plan["d_owner"] == c
        wi = plan["w"][m]
        k = plan["rank"][m]
        gi = gidx_all[m]
        dl = plan["d_local"][m]
        we = ew[m]
        # position within run: order by (w, k, dst)
        order = np.lexsort((dl, k, wi))
        wi, k, gi, dl, we = (wi[order], k[order], gi[order], dl[order],
                             we[order])
        runkey = wi * plan["KMAX"] + k
        new = np.ones(len(runkey), dtype=bool)
        new[1:] = runkey[1:] != runkey[:-1]
        idxs = np.arange(len(runkey))
        starts = idxs[new]
        pos = idxs - starts[np.cumsum(new) - 1]
        slot = plan["run_off"][wi, k] + pos

        gidx16 = np.zeros(NSLOT, dtype=np.int16)
        sidx16 = np.full(NSLOT, JUNK, dtype=np.int16)
        ewb = np.zeros(NSLOT, dtype=np.float32)
        gidx16[slot] = gi.astype(np.int16)
        sidx16[slot] = (dl + (k % 2) * AGGR).astype(np.int16)
        ewb[slot] = we

        # combined per-segment blocks: [gather idx | scatter idx]
        cidx = np.zeros(2 * NSLOT, dtype=np.int16)
        for segs in plan["segments"]:
            for (soff, nsl, _runs) in segs:
                cidx[2 * soff:2 * soff + nsl] = gidx16[soff:soff + nsl]
                cidx[2 * soff + nsl:2 * (soff + nsl)] = \
                    sidx16[soff:soff + nsl]

        per_core.append(dict(
            c_idx=cidx.reshape(2 * NSLOT // 16, 16).T.copy(),
            ew_s=ewb.reshape(NSLOT // 128, 128).T.astype(bf),
        ))

    for c in range(NCORES):
        xs = np.zeros((cfg["IN_C"], NPAD), dtype=np.float32)
        xs[:, :SHARD] = np.asarray(x[c * SHARD:(c + 1) * SHARD]).T
        per_core[c]["x_t"] = xs.astype(bf)
    return per_core


def _build(cfg, plan):
    IN_C, HID, NCLS = cfg["IN_C"], cfg["HID"], cfg["NCLS"]
    NPAD, NTILES = cfg["NPAD"], cfg["NTILES"]
    NTOT, WINR, NWIN = cfg["NTOT"], cfg["WINR"], cfg["NWIN"]
    NSLOT = plan["NSLOT"]
    RL, run_off, segments = plan["RL"], plan["run_off"], plan["segments"]
    NPAIR = NPAD // 2
    AGGR = NPAD + 128
    NPADDING = float(NCORES * NPAD - cfg["N"])
    INVN = 1.0 / cfg["N"]
    SEGC = SEGMAX // 128
    LVL = int(os.environ.get("KLEVEL", "0"))
    NCH = NSLOT // 128

    nc = bacc.Bacc("TRN2", target_bir_lowering=False, debug=False,
                   num_devices=NCORES)

    def din(name, shape, dt):
        return nc.dram_tensor(name, shape, dt, kind="ExternalInput").ap()

    x_t_d = din("x_t", [IN_C, NPAD], BF16)
    cidx_d = din("c_idx", [16, 2 * NSLOT // 16], I16)
    ew_d = din("ew_s", [128, NCH], BF16)
    # packed weights: wb = bf16 [128, 64+64+64+128+64+5=389]
    # (cw0 | cw1dup | cw2dup | mw1dup | mw2 | mw3)
    # wf = f32 [128, 16]: cols 0-5 pad_corr(x3 layers)+junk rows, 6-8 cbd,
    # 9-11 bng, 12-14 bnb (HID rows), 15: mb1(128) ; mb2/mb3 packed in
    # rows of col 15? -> keep separate small cols; see host packing.
    wb_d = din("w_bf", [128, 389], BF16)
    wf_d = din("w_f32", [128, 20], F32)
    out_d = nc.dram_tensor("out5", [NCLS, NPAD], F32,
                           kind="ExternalOutput").ap()
    dbg = bool(os.environ.get("KDBG"))
    if dbg:
        dbg_tab = nc.dram_tensor("dbg_tab", [NTOT, 64], F32,
                                 kind="ExternalOutput").ap()
        dbg_agg = nc.dram_tensor("dbg_agg", [NPAD, 64], F32,
                                 kind="ExternalOutput").ap()
        dbg_h = nc.dram_tensor("dbg_h", [128, NPAIR], F32,
                               kind="ExternalOutput").ap()

    rg = [list(range(NCORES))]

    with tile.TileContext(nc) as tc:
        with tc.tile_pool(name="sb", bufs=1) as sb, \
             tc.tile_pool(name="sb2", bufs=2) as sb2, \
             tc.tile_pool(name="hxp", bufs=2) as hxp, \
             tc.tile_pool(name="drp", bufs=1) as drp, \
             tc.tile_pool(name="mp", bufs=1) as mp, \
             tc.tile_pool(name="idxp", bufs=2) as idxp, \
             tc.tile_pool(name="psP", bufs=2, space="PSUM") as psP, \
             tc.tile_pool(name="psM", bufs=1, space="PSUM") as psM, \
             tc.tile_pool(name="dram", bufs=1, space="DRAM") as dram, \
             tc.tile_pool(name="dram2", bufs=2, space="DRAM") as dram2:

            # ---- persistent loads ----
            ew_f = sb.tile([128, NCH], F32, tag="ewf")
            EWCH = 1664
            for e0 in range(0, NCH, EWCH):
                e1 = min(e0 + EWCH, NCH)
                ewt = sb2.tile([128, EWCH], BF16, tag="ewtmp")
                nc.sync.dma_start(out=ewt[:, 0:e1 - e0], in_=ew_d[:, e0:e1])
                nc.vector.tensor_copy(out=ew_f[:, e0:e1],
                                      in_=ewt[:, 0:e1 - e0])
            wb = sb.tile([128, 389], BF16, tag="wb")
            nc.sync.dma_start(out=wb[:], in_=wb_d[:])
            wf = sb.tile([128, 20], F32, tag="wf")
            nc.sync.dma_start(out=wf[:], in_=wf_d[:])
            cw_t = [wb[:, 0:64], wb[:, 64:128], wb[:, 128:192]]
            mw1_t = wb[:, 192:320]
            mw2_t = wb[0:2 * HID, 320:384]
            mw3_t = wb[0:HID, 384:389]
            pcorr_t = wf[0:HID, 0:6]
            cb2_t = [wf[:, 6 + l:7 + l] for l in range(3)]
            bng_t = [wf[0:HID, 9 + l:10 + l] for l in range(3)]
            bnb_t = [wf[0:HID, 12 + l:13 + l] for l in range(3)]
            mb1_t = wf[:, 15:16]
            mb2_t = wf[0:HID, 16:17]
            mb3_t = wf[0:NCLS, 17:18]
            eps_t = wf[0:HID, 18:19]
            zt = sb.tile([128, 1664], F32, tag="zero")
            nc.vector.memset(zt[:], 0.0)

            x_t = sb.tile([IN_C, NPAD], BF16, tag="hx0")
            nc.sync.dma_start(out=x_t[:], in_=x_t_d[:])

            # replicate wrapped indices [16, X] -> [128, X] in DRAM
            cidx_rep = dram.tile([128, 2 * NSLOT // 16], I16, tag="cidxrep")
            if LVL < 3:
                for r in range(8):
                    nc.sync.dma_start(
                        out=cidx_rep[16 * r:16 * (r + 1), :], in_=cidx_d[:])
            # pre-zeroed DRAM block to reset the aggregate each layer
            zdram = dram.tile([2 * AGGR, 64], F32, tag="zdram")
            zv = zdram[:].rearrange("(k p) f -> p k f", p=128)
            NZCH = 2 * AGGR // 128
            ZC = 1664 // 64
            for z0 in range(0, NZCH, ZC):
                z1 = min(z0 + ZC, NZCH)
                nc.sync.dma_start(
                    out=zv[:, z0:z1, :],
                    in_=zt[:, 0:(z1 - z0) * 64].rearrange(
                        "p (k f) -> p k f", f=64))

            h_cur = x_t              # layer0: plain feature-major
            for l in range(3):
                cdim = IN_C if l == 0 else HID
                # ---- projection -> compact node-major f32 table ----
                tab_in = dram2.tile([NPAD, 64], F32, tag="tabin")
                PB = 16
                if l == 0:
                    srcs = [(h_cur, 0, NTILES, tab_in[:])]
                else:
                    evens = tab_in[:].rearrange("(q i) f -> q i f", i=2)
                    srcs = [(h_cur, 0, NTILES // 2, evens[:, 0, :]),
                            (h_cur, HID, NTILES // 2, evens[:, 1, :])]
                for hsrc, prow, ntile, tview in srcs:
                    for g in range(0, ntile, PB):
                        gn = min(PB, ntile - g)
                        pp = psP.tile([128, PB * HID], F32, tag="proj",
                                      space="PSUM")
                        for j in range(gn):
                            kk = g + j
                            nc.tensor.matmul(
                                out=pp[:, j * HID:(j + 1) * HID],
                                lhsT=hsrc[prow:prow + cdim,
                                          kk * 128:(kk + 1) * 128],
                                rhs=cw_t[l][prow:prow + cdim, :],
                                start=True, stop=True)
                        stg = sb2.tile([128, PB, HID], F32, tag="stage")
                        nc.any.tensor_copy(
                            out=stg[:, 0:gn, :],
                            in_=pp[:, 0:gn * HID].rearrange(
                                "p (j f) -> p j f", j=gn))
                        tv = tview.rearrange("(g p) f -> g p f", p=128)
                        nc.sync.dma_start(
                            out=tv[g:g + gn].rearrange("g p f -> p g f"),
                            in_=stg[:, 0:gn, :])

                # ---- AllGather the projected table ----
                tab_full = dram2.tile([NTOT, 64], F32, tag="tabfull")
                if LVL < 4:
                    nc.gpsimd.collective_compute(
                        "AllGather", ALU.bypass, replica_groups=rg,
                        ins=[tab_in[:]], outs=[tab_full[:]])

                # ---- zero aggregate, gather + scatter-add ----
                agg = dram2.tile([2 * AGGR, 64], F32, tag="agg")
                nc.sync.dma_start(out=agg[:], in_=zdram[:])
                for wi in range(NWIN):
                    for (soff, nsl, runs) in segments[wi]:
                        segc = nsl // 128
                        if LVL < 3:
                            ci_t = idxp.tile([128, 2 * SEGC * 8], I16,
                                             tag="ci")
                            nc.sync.dma_start(
                                out=ci_t[:, 0:2 * nsl // 16],
                                in_=cidx_rep[:, 2 * soff // 16:
                                             2 * (soff + nsl) // 16])
                            gi_t = ci_t[:, 0:nsl // 16]
                            si_t = ci_t[:, nsl // 16:2 * nsl // 16]
                            m_t = mp.tile([128, SEGC, 64], F32, tag="mt")
                            nc.gpsimd.dma_gather(
                                out_ap=m_t[:, 0:segc, :],
                                in_ap=tab_full[wi * WINR:(wi + 1) * WINR, :],
                                idxs_ap=gi_t,
                                num_idxs=nsl, num_idxs_reg=nsl,
                                elem_size=64, single_packet=False)
                        if LVL < 2:
                            a0, a1 = bass.broadcast_tensor_aps(
                                m_t[:, 0:segc, :],
                                ew_f[:, soff // 128:soff // 128 + segc]
                                .rearrange("p (k a) -> p k a", a=1))
                            nc.vector.tensor_tensor(
                                out=m_t[:, 0:segc, :], in0=a0, in1=a1,
                                op=ALU.mult)
                        if LVL < 1:
                            # pair-merged conflict-free spans (k//2 groups)
                            spans = []
                            for k in runs:
                                ro, rl = int(run_off[wi][k]), int(RL[wi][k])
                                if spans and spans[-1][2] == k // 2 and \
                                        spans[-1][1] == ro:
                                    spans[-1] = (spans[-1][0], ro + rl,
                                                 k // 2)
                                else:
                                    spans.append((ro, ro + rl, k // 2))
                            for (a, b, _pid) in spans:
                                for p0 in range(a, b, SCATMAX):
                                    pn = min(SCATMAX, b - p0)
                                    lo = p0 - soff
                                    nc.gpsimd.dma_scatter_add(
                                        out_ap=agg[:],
                                        in_ap=m_t[:, lo // 128:
                                                  (lo + pn) // 128, :],
                                        idxs_ap=si_t[:, lo // 16:
                                                     (lo + pn) // 16],
                                        num_idxs=pn, num_idxs_reg=pn,
                                        elem_size=64, single_packet=False)

                # ---- drain: f32 node-major -> bf16 pair-rows -> xbar T ----
                if dbg and l == 0:
                    nc.sync.dma_start(out=dbg_tab[:], in_=tab_full[:])
                    nc.sync.dma_start(out=dbg_agg[:], in_=agg[0:NPAD, :])  # even half only
                af = drp.tile([128, NTILES, 64], F32, tag="af")
                av0 = agg[0:NPAD, :].rearrange("(k p) f -> p k f", p=128)
                av1 = agg[AGGR:AGGR + NPAD, :].rearrange(
                    "(k p) f -> p k f", p=128)
                nc.sync.dma_start(out=af[:], in_=av0)
                HT = NTILES // 2
                for hh in range(2):
                    nc.gpsimd.dma_start(
                        out=af[:, hh * HT:(hh + 1) * HT, :],
                        in_=av1[:, hh * HT:(hh + 1) * HT, :],
                        accum_op=ALU.add)
                abf = drp.tile([128, NTILES, 64], BF16, tag="abf")
                nc.vector.tensor_copy(out=abf[:], in_=af[:])
                tmp = dram2.tile([NPAIR, 128], BF16, tag="ptmp")
                nc.sync.dma_start(
                    out=tmp[:].rearrange("(k q) (i f) -> (q i) k f",
                                         q=64, i=2),
                    in_=abf[:])
                h_new = hxp.tile([128, NPAIR], BF16, tag="hx")
                nc.sync.dma_start_transpose(out=h_new[:], in_=tmp[:])
                # bias + relu (bias duplicated across parity halves)
                nc.vector.tensor_scalar(
                    out=h_new[:], in0=h_new[:], scalar1=cb2_t[l],
                    scalar2=0.0, op0=ALU.add, op1=ALU.max)

                # ---- BN stats (global, pad-row corrected) ----
                scr = drp.tile([128, NPAIR], BF16, tag="scr")
                st2 = sb.tile([128, 2], F32, tag="st2")
                nc.scalar.activation(out=scr[:], in_=h_new[:],
                                     func=ACTF.Copy, accum_out=st2[:, 0:1])
                nc.scalar.activation(out=scr[:], in_=h_new[:],
                                     func=ACTF.Square, accum_out=st2[:, 1:2])
                fold = sb.tile([HID, 2], F32, tag="fold")
                nc.sync.dma_start(out=fold[:], in_=st2[HID:128, :])
                sc = sb.tile([HID, 2], F32, tag="statsc")
                nc.vector.tensor_tensor(out=sc[:], in0=pcorr_t[:, 2 * l:
                                                              2 * l + 2],
                                        in1=st2[0:HID, :], op=ALU.add)
                nc.vector.tensor_tensor(out=sc[:], in0=sc[:],
                                        in1=fold[:], op=ALU.add)
                stats_in = dram.tile([HID, 2], F32, tag=f"stin{l}")
                stats_out = dram.tile([HID, 2], F32, tag=f"stout{l}")
                st_sb = sb.tile([HID, 2], F32, tag="stsb")
                if LVL < 4:
                    nc.sync.dma_start(out=stats_in[:], in_=sc[:])
                    nc.gpsimd.collective_compute(
                        "AllReduce", ALU.add, replica_groups=rg,
                        ins=[stats_in[:]], outs=[stats_out[:]])
                    nc.sync.dma_start(out=st_sb[:], in_=stats_out[:])
                else:
                    nc.vector.tensor_copy(out=st_sb[:], in_=sc[:])
                mt = sb.tile([HID, 1], F32, tag="mt1")
                nc.vector.tensor_scalar(out=mt[:], in0=st_sb[:, 0:1],
                                        scalar1=INVN, scalar2=None,
                                        op0=ALU.mult)
                vt = sb.tile([HID, 1], F32, tag="vt")
                nc.vector.tensor_scalar(out=vt[:], in0=st_sb[:, 1:2],
                                        scalar1=INVN, scalar2=None,
                                        op0=ALU.mult)
                msq = sb.tile([HID, 1], F32, tag="msq")
                nc.vector.tensor_tensor(out=msq[:], in0=mt[:], in1=mt[:],
                                        op=ALU.mult)
                nc.vector.tensor_tensor(out=vt[:], in0=vt[:], in1=msq[:],
                                        op=ALU.subtract)
                sqv = sb.tile([HID, 1], F32, tag="sqv")
                nc.scalar.activation(out=sqv[:], in_=vt[:], func=ACTF.Sqrt,
                                     bias=eps_t)
                rstd = sb.tile([HID, 1], F32, tag="rstd")
                nc.vector.reciprocal(out=rstd[:], in_=sqv[:])
                s_t = sb.tile([128, 1], F32, tag="sT")
                nc.vector.tensor_tensor(out=s_t[0:HID, :], in0=bng_t[l],
                                        in1=rstd[:], op=ALU.mult)
                t_t = sb.tile([128, 1], F32, tag="tT")
                nc.vector.tensor_tensor(out=t_t[0:HID, :], in0=mt[:],
                                        in1=s_t[0:HID, :], op=ALU.mult)
                nc.vector.tensor_tensor(out=t_t[0:HID, :], in0=bnb_t[l],
                                        in1=t_t[0:HID, :], op=ALU.subtract)
                nc.sync.dma_start(out=s_t[HID:128, :], in_=s_t[0:HID, :])
                nc.sync.dma_start(out=t_t[HID:128, :], in_=t_t[0:HID, :])
                nc.vector.tensor_scalar(out=h_new[:], in0=h_new[:],
                                        scalar1=s_t[:], scalar2=t_t[:],
                                        op0=ALU.mult, op1=ALU.add)
                if dbg and l == 0:
                    dh1 = drp.tile([128, NPAIR], F32, tag="dh1")
                    nc.vector.tensor_copy(out=dh1[:], in_=h_new[:])
                    nc.sync.dma_start(out=dbg_h[:], in_=dh1[:])
                h_cur = h_new

            # ---- MLP head (parity-interleaved feature-major) ----
            MB = 512
            ov = out_d[:].rearrange("c (q i) -> c q i", i=2)
            for par in range(2):
                hsec = h_cur[par * HID:(par + 1) * HID, :]
                for s0 in range(0, NPAIR, MB):
                    s1 = min(s0 + MB, NPAIR)
                    n = s1 - s0
                    p1 = psM.tile([2 * HID, MB], F32, tag="mp1", space="PSUM")
                    nc.tensor.matmul(out=p1[:, 0:n],
                                     lhsT=mw1_t[par * HID:(par + 1) * HID, :],
                                     rhs=hsec[:, s0:s1],
                                     start=True, stop=True)
                    a1 = sb2.tile([2 * HID, MB], BF16, tag="a1")
                    nc.scalar.activation(out=a1[:, 0:n], in_=p1[:, 0:n],
                                         func=ACTF.Gelu, bias=mb1_t)
                    p2 = psM.tile([HID, MB], F32, tag="mp2", space="PSUM")
                    nc.tensor.matmul(out=p2[:, 0:n], lhsT=mw2_t,
                                     rhs=a1[:, 0:n], start=True, stop=True)
                    a2 = sb2.tile([HID, MB], BF16, tag="a2")
                    nc.scalar.activation(out=a2[:, 0:n], in_=p2[:, 0:n],
                                         func=ACTF.Gelu, bias=mb2_t)
                    p3 = psM.tile([NCLS, MB], F32, tag="mp3", space="PSUM")
                    nc.tensor.matmul(out=p3[:, 0:n], lhsT=mw3_t,
                                     rhs=a2[:, 0:n], start=True, stop=True)
                    ob = sb2.tile([NCLS, MB], F32, tag="ob")
                    nc.vector.tensor_scalar(out=ob[:, 0:n], in0=p3[:, 0:n],
                                            scalar1=mb3_t, scalar2=None,
                                            op0=ALU.add)
                    nc.sync.dma_start(out=ov[:, s0:s1, par],
                                      in_=ob[:, 0:n])
    nc.compile()
    return nc


def kernel(x, edge_index, edge_attr,
           conv_w0, conv_b0, conv_w1, conv_b1, conv_w2, conv_b2,
           bn_g0, bn_be0, bn_g1, bn_be1, bn_g2, bn_be2,
           mlp_w1, mlp_b1, mlp_w2, mlp_b2, mlp_w3, mlp_b3):
    x = np.asarray(x)
    N, in_c = x.shape
    hid = np.asarray(conv_w0).shape[1]
    ncls = np.asarray(mlp_w3).shape[1]
    cfg = _cfg(N, in_c, hid, ncls)

    edge_index = np.asarray(edge_index)
    plan = _plan(edge_index, cfg)
    per_core = _preprocess(x, edge_index, edge_attr, cfg, plan)

    bf = ml_dtypes.bfloat16
    EPS_ = EPS
    wbp = np.zeros((128, 389), np.float32)
    wbp[:, 0:64] = np.asarray(conv_w0)
    wbp[0:hid, 64:128] = np.asarray(conv_w1)
    wbp[hid:128, 64:128] = np.asarray(conv_w1)
    wbp[0:hid, 128:192] = np.asarray(conv_w2)
    wbp[hid:128, 128:192] = np.asarray(conv_w2)
    wbp[0:hid, 192:320] = np.asarray(mlp_w1)
    wbp[hid:128, 192:320] = np.asarray(mlp_w1)
    wbp[0:2 * hid, 320:384] = np.asarray(mlp_w2)
    wbp[0:hid, 384:389] = np.asarray(mlp_w3)
    npadding = float(NCORES * cfg["NPAD"] - N)
    wfp = np.zeros((128, 20), np.float32)
    for l, (cb, bg, be) in enumerate([(conv_b0, bn_g0, bn_be0),
                                      (conv_b1, bn_g1, bn_be1),
                                      (conv_b2, bn_g2, bn_be2)]):
        cb = np.asarray(cb, dtype=np.float32)
        pb = np.maximum(cb, 0.0)
        wfp[0:hid, 2 * l] = -npadding * pb
        wfp[0:hid, 2 * l + 1] = -npadding * pb * pb
        wfp[0:hid, 6 + l] = cb
        wfp[hid:128, 6 + l] = cb
        wfp[0:hid, 9 + l] = np.asarray(bg)
        wfp[0:hid, 12 + l] = np.asarray(be)
    wfp[:, 15] = np.asarray(mlp_b1, dtype=np.float32)
    wfp[0:hid, 16] = np.asarray(mlp_b2, dtype=np.float32)
    wfp[0:ncls, 17] = np.asarray(mlp_b3, dtype=np.float32)
    wfp[0:hid, 18] = EPS_
    common = dict(w_bf=wbp.astype(bf), w_f32=wfp)
    in_maps = []
    for c in range(NCORES):
        m = dict(common)
        m["x_t"] = per_core[c]["x_t"]
        m["c_idx"] = per_core[c]["c_idx"]
        m["ew_s"] = per_core[c]["ew_s"]
        in_maps.append(m)

    nc = _build(cfg, plan)
    nc.m = get_hw_module(nc.m)
    res = bass_utils.run_bass_kernel_spmd(
        nc, in_maps, core_ids=list(range(NCORES)))

    kernel._last_res = res
    kernel._last_nc = nc
    kernel._last_in_maps = in_maps
    out = np.empty((N, cfg["NCLS"]), dtype=np.float32)
    SHARD = cfg["SHARD"]
    for c in range(NCORES):
        out[c * SHARD:(c + 1) * SHARD] = res.results[c]["out5"][:, :SHARD].T
    return out
